# revision 7
# baseline (speedup 1.0000x reference)
"""GCN layer kernel for Trainium2 (8 NeuronCores).

Strategy:
  - Nodes assigned to 8 cores x BPC blocks of 128 via balanced packing so
    each (block, src-quarter) edge segment fits C_q*128 edges (minimal pad).
  - Edge gather: x rows fetched from HBM with gpsimd.dma_gather (SWDGE),
    <=1024 indices per call (HW limit), calls spread over 4 SWDGE queues
    (parallel Q7 descriptor-gen pairs). int16 indices reach 32767 rows, so
    x is split into 4 quarters with shifted base APs.
  - Segment-sum: per 128-edge chunk, one-hot S[e, slot] built on DVE
    (is_equal vs iota, batched per block), PE matmul accumulates
    psum[feat, slot] += E_bf16^T S over the block's chunks. Pad edges get
    slot 128 (PSUM cols 128..131 ignored).
  - mean+linear+relu+residual: psum1 -> SBUF (ACT copy), f32 matmul with
    W^T, ACT relu with per-partition scale 1/deg (folds the mean), DVE
    residual add, DMA out. deg==0 nodes get a self-edge and scale 1 so
    they keep x (DGL semantics).
"""
import sys
sys.path.insert(0, "/opt/trn_rl_repo")

import numpy as np
import ml_dtypes

import concourse.bass as bass
import concourse.mybir as mybir
import concourse.tile as tile
from concourse import bacc, bass_utils

F32 = mybir.dt.float32
BF16 = mybir.dt.bfloat16
I16 = mybir.dt.int16

N_NODES = 100000
D = 128
NCORES = 8
SW = 132  # slot one-hot width (128 real slots + pad cols)
PAD_SLOT = 128
GATHER_CAP = 1024  # dma_gather crashes above 1024 indices per call

_BUILD_CACHE = {}
LAST_RESULTS = None  # for test harness introspection


def _plan_groups(bpc, gmax):
    groups = []
    left = bpc
    while left > 0:
        g = min(gmax, left)
        groups.append(g)
        left -= g
    return groups


def _build(c_list, groups, bpc, npad, qrows):
    """Build + compile the SPMD Bass program. Same program for all 8 cores."""
    key = (tuple(c_list), tuple(groups), bpc, npad, qrows)
    if key in _BUILD_CACHE:
        return _BUILD_CACHE[key]

    csum = int(sum(c_list))
    totslots = bpc * csum * 128
    npc = bpc * 128  # nodes per core

    nc = bacc.Bacc("TRN2", target_bir_lowering=False, debug=False,
                   num_devices=NCORES, num_swdge_queues=4)
    xq = nc.dram_tensor("xq", [npad, D], BF16, kind="ExternalInput")
    idxd = nc.dram_tensor("idxd", [128, totslots // 16], I16,
                          kind="ExternalInput")
    slotd = nc.dram_tensor("slotd", [128, bpc * csum], BF16,
                           kind="ExternalInput")
    invd = nc.dram_tensor("invd", [128, bpc], F32, kind="ExternalInput")
    xod = nc.dram_tensor("xod", [npc, D], F32, kind="ExternalInput")
    wtd = nc.dram_tensor("wtd", [D, D], F32, kind="ExternalInput")
    iotad = nc.dram_tensor("iotad", [128, SW], BF16, kind="ExternalInput")
    outd = nc.dram_tensor("out", [npc, D], F32, kind="ExternalOutput")

    qcall = 0  # rotates gather calls over the 4 SWDGE queues

    with tile.TileContext(nc) as tc:
        with tc.tile_pool(name="const", bufs=1) as const, \
             tc.tile_pool(name="ework", bufs=3) as ework, \
             tc.tile_pool(name="sbwork", bufs=3) as sbwork, \
             tc.tile_pool(name="psum1", bufs=2, space="PSUM") as psum1p, \
             tc.tile_pool(name="psum2", bufs=2, space="PSUM") as psum2p:

            wt_t = const.tile([128, D], F32)
            nc.sync.dma_start(out=wt_t[:], in_=wtd[:, :])
            iota_t = const.tile([128, SW], BF16)
            nc.sync.dma_start(out=iota_t[:], in_=iotad[:, :])
            inv_t = const.tile([128, bpc], F32)
            nc.sync.dma_start(out=inv_t[:], in_=invd[:, :])

            base_col = 0
            b0 = 0
            for g_i, G in enumerate(groups):
                slot_t = ework.tile([128, G * csum], BF16, tag="slot")
                nc.sync.dma_start(
                    out=slot_t[:],
                    in_=slotd[:, b0 * csum:(b0 + G) * csum])
                eb_list = []
                for q in range(4):
                    nidx = G * c_list[q] * 128
                    ncols = nidx // 16
                    idx_t = ework.tile([128, ncols], I16, tag=f"idx{q}")
                    nc.sync.dma_start(
                        out=idx_t[:],
                        in_=idxd[:, base_col:base_col + ncols])
                    ef = ework.tile([128, G * c_list[q], 128], BF16,
                                    tag=f"ef{q}")
                    c0 = 0
                    while c0 * 128 < nidx:
                        n_call = min(GATHER_CAP, nidx - c0 * 128)
                        nch = n_call // 128
                        nc.gpsimd.dma_gather(
                            out_ap=ef[:, c0:c0 + nch, :],
                            in_ap=xq[q * qrows:, :],
                            idxs_ap=idx_t[:, c0 * 8:(c0 + nch) * 8],
                            num_idxs=n_call,
                            num_idxs_reg=n_call,
                            elem_size=D,
                            queue_num=qcall % 4,
                        )
                        qcall += 1
                        c0 += nch
                    eb_list.append(ef)
                    base_col += ncols

                for bl in range(G):
                    b = b0 + bl
                    s_t = ework.tile([128, csum, SW], BF16, tag="S")
                    slot_sl = slot_t[:, bl * csum:(bl + 1) * csum]
                    nc.vector.tensor_tensor(
                        out=s_t[:, :, :],
                        in0=slot_sl.unsqueeze(2).to_broadcast(
                            [128, csum, SW]),
                        in1=iota_t[:].unsqueeze(1).to_broadcast(
                            [128, csum, SW]),
                        op=mybir.AluOpType.is_equal,
                    )
                    p1 = psum1p.tile([128, SW], F32, tag="p1")
                    cglob = 0
                    for q in range(4):
                        cq = c_list[q]
                        for k in range(cq):
                            nc.tensor.matmul(
                                out=p1[:, :],
                                lhsT=eb_list[q][:, bl * cq + k, :],
                                rhs=s_t[:, cglob, :],
                                start=(cglob == 0),
                                stop=(cglob == csum - 1),
                            )
                            cglob += 1
                    agg_t = sbwork.tile([128, 128], F32, tag="aggT")
                    nc.scalar.copy(agg_t[:], p1[:, 0:128])
                    p2 = psum2p.tile([128, 128], F32, tag="p2")
                    nc.tensor.matmul(out=p2[:, :], lhsT=agg_t[:],
                                     rhs=wt_t[:], start=True, stop=True)
                    xo_t = sbwork.tile([128, 128], F32, tag="xo")
                    nc.scalar.dma_start(out=xo_t[:],
                                        in_=xod[b * 128:(b + 1) * 128, :])
                    hb_t = sbwork.tile([128, 128], F32, tag="hb")
                    nc.scalar.activation(
                        hb_t[:], p2[:, :],
                        mybir.ActivationFunctionType.Relu,
                        scale=inv_t[:, b:b + 1])
                    ob_t = sbwork.tile([128, 128], F32, tag="ob")
                    nc.vector.tensor_add(ob_t[:], hb_t[:], xo_t[:])
                    nc.scalar.dma_start(out=outd[b * 128:(b + 1) * 128, :],
                                        in_=ob_t[:])
                b0 += G
    nc.compile()
    _BUILD_CACHE[key] = nc
    return nc


def _pack_blocks(qd, nblocks, cap, node_cap=128):
    """Greedy 4-D balanced packing: assign nodes to blocks so that each
    block's per-quarter edge counts stay <= cap and node count <= node_cap.
    qd: [n, 4] per-node per-quarter in-degree. Returns block id per node,
    or None if infeasible."""
    n = qd.shape[0]
    loads = np.zeros((nblocks, 4), dtype=np.int64)
    slots = np.full(nblocks, node_cap, dtype=np.int64)
    assign = np.full(n, -1, dtype=np.int64)
    order = np.argsort(-qd.sum(1), kind="stable")
    # big nodes first with exact argmin; tail nodes in bulk round-robin
    big = order[qd[order].sum(1) > 24]
    small = order[qd[order].sum(1) <= 24]
    for i in big:
        score = (loads + qd[i]).max(1)
        score[slots <= 0] = 1 << 40
        score[(loads + qd[i] > cap).any(1)] = 1 << 40
        b = int(np.argmin(score))
        if score[b] >= 1 << 40:
            return None
        assign[i] = b
        loads[b] += qd[i]
        slots[b] -= 1
    # small nodes: repeatedly place into least-loaded blocks
    for i in small:
        score = (loads + qd[i]).max(1).astype(np.float64)
        score += (node_cap - slots) * 1e-3  # prefer emptier blocks slightly
        score[slots <= 0] = 1e18
        score[(loads + qd[i] > cap).any(1)] = 1e18
        b = int(np.argmin(score))
        if score[b] >= 1e18:
            return None
        assign[i] = b
        loads[b] += qd[i]
        slots[b] -= 1
    return assign


def _preprocess(x, src, dst, W, n_nodes, ncores, gmax=4, bpc=None):
    """Host-side graph partitioning -> per-core tensors + chunk budgets."""
    D_ = x.shape[1]
    if bpc is None:
        # extra blocks beyond the minimum give the packer slack
        min_bpc = -(-n_nodes // (ncores * 128))
        bpc = min_bpc + 2 if min_bpc > 8 else min_bpc
    npc = bpc * 128
    npad = npc * ncores
    qrows = npad // 4
    nblocks = ncores * bpc

    deg = np.bincount(dst, minlength=n_nodes)
    inv = 1.0 / np.maximum(deg, 1).astype(np.float32)
    zero_deg = np.where(deg == 0)[0]
    if len(zero_deg):
        src = np.concatenate([src, zero_deg])
        dst = np.concatenate([dst, zero_deg])

    quarter = (src // qrows).astype(np.int64)

    # per-node per-quarter in-degree -> balanced packing
    qd = np.zeros((npad, 4), dtype=np.int64)
    np.add.at(qd, (dst, quarter), 1)
    assign = _pack_blocks(qd, nblocks, cap=512)
    if assign is not None:
        c_list = [4, 4, 4, 4]
    else:
        # fallback: natural-order blocks, budgets from data
        assign = np.arange(npad) // 128
        cmat = np.zeros((nblocks, 4), dtype=np.int64)
        np.add.at(cmat, (assign[dst], quarter), 1)
        c_list = [int(-(-cmat[:, q].max() // 128)) for q in range(4)]
        c_list = [max(c, 1) for c in c_list]
    csum = int(sum(c_list))
    qoff = np.concatenate([[0], np.cumsum(c_list)]).astype(int)

    # node -> (block, slot); slot = rank within block
    order_nodes = np.argsort(assign[:npad], kind="stable")
    # nodes with assign==-1 (padding, unassigned) -> fill remaining slots
    blk_of = assign[:npad].copy()
    unassigned = np.where(blk_of < 0)[0]
    if len(unassigned):
        counts = np.bincount(blk_of[blk_of >= 0], minlength=nblocks)
        free = []
        for b in range(nblocks):
            free.extend([b] * (128 - counts[b]))
        blk_of[unassigned] = np.array(free[:len(unassigned)], dtype=np.int64)
    order_nodes = np.argsort(blk_of, kind="stable")
    slot_of = np.zeros(npad, dtype=np.int64)
    counts = np.bincount(blk_of, minlength=nblocks)
    assert counts.max() <= 128, "block overflow"
    start = np.concatenate([[0], np.cumsum(counts)])
    slot_of[order_nodes] = np.arange(npad) - start[blk_of[order_nodes]]
    # perm[i] = node occupying padded position i (core-major, block, slot)
    pos_of = blk_of * 128 + slot_of
    perm = np.zeros(npad, dtype=np.int64)
    perm[pos_of] = np.arange(npad)

    groups = _plan_groups(bpc, gmax)
    call_base = np.zeros((len(groups), 4), dtype=np.int64)
    pos = 0
    for gi, G in enumerate(groups):
        for q in range(4):
            call_base[gi, q] = pos
            pos += G * c_list[q] * 128
    totslots = pos
    assert totslots == bpc * csum * 128

    g_of_block = np.zeros(bpc, dtype=np.int64)
    boff_of_block = np.zeros(bpc, dtype=np.int64)
    b = 0
    for gi, G in enumerate(groups):
        for j in range(G):
            g_of_block[b] = gi
            boff_of_block[b] = j
            b += 1

    # per-edge data
    blk_e = blk_of[dst]
    slot_e = slot_of[dst]
    order = np.lexsort((src, quarter, blk_e))
    src_s = src[order]
    q_s = quarter[order]
    blk_s = blk_e[order]
    slot_s = slot_e[order]

    seg_id = blk_s * 4 + q_s
    seg_counts = np.bincount(seg_id, minlength=nblocks * 4)
    cmat = seg_counts.reshape(nblocks, 4)
    for q in range(4):
        assert cmat[:, q].max() <= c_list[q] * 128, \
            f"quarter {q} overflow: {cmat[:, q].max()}"
    seg_start = np.concatenate([[0], np.cumsum(seg_counts)])
    rank = np.arange(len(src_s)) - seg_start[seg_id]
    core_e = blk_s // bpc
    bl_local = blk_s % bpc
    cq_e = np.array(c_list)[q_s]
    pos_e = (call_base[g_of_block[bl_local], q_s]
             + boff_of_block[bl_local] * cq_e * 128 + rank)

    idx16 = np.zeros((ncores, 16, totslots // 16), dtype=np.int16)
    idx16[core_e, pos_e % 16, pos_e // 16] = (src_s - q_s * qrows).astype(
        np.int16)
    idx_rep = np.tile(idx16, (1, 8, 1))

    slot_arr = np.full((ncores, 128, bpc * csum), PAD_SLOT, dtype=np.float32)
    chunk_in_block = qoff[q_s] + rank // 128
    slot_arr[core_e, rank % 128, bl_local * csum + chunk_in_block] = \
        slot_s.astype(np.float32)
    slot_bf = slot_arr.astype(ml_dtypes.bfloat16)

    inv_arr = np.ones((ncores, 128, bpc), dtype=np.float32)
    nodes = np.arange(n_nodes)
    inv_arr[blk_of[nodes] // bpc, slot_of[nodes], blk_of[nodes] % bpc] = inv

    xpad = np.zeros((npad, D_), dtype=np.float32)
    xpad[:n_nodes] = x
    xpad_bf = xpad.astype(ml_dtypes.bfloat16)
    # xod rows ordered by padded position: row (core, block, slot) = x[node]
    xperm = xpad[perm]

    iota = np.tile(np.arange(SW, dtype=np.float32)[None, :],
                   (128, 1)).astype(ml_dtypes.bfloat16)
    wt = np.ascontiguousarray(W.T.astype(np.float32))

    in_maps = []
    for c in range(ncores):
        in_maps.append({
            "xq": xpad_bf,
            "idxd": np.ascontiguousarray(idx_rep[c]),
            "slotd": np.ascontiguousarray(slot_bf[c]),
            "invd": np.ascontiguousarray(inv_arr[c]),
            "xod": np.ascontiguousarray(xperm[c * npc:(c + 1) * npc]),
            "wtd": wt,
            "iotad": iota,
        })
    return in_maps, c_list, groups, bpc, npad, qrows, perm


def kernel(x, src, dst, W, n_nodes=None, trace=False):
    global LAST_RESULTS
    x = np.ascontiguousarray(np.asarray(x, dtype=np.float32))
    W = np.ascontiguousarray(np.asarray(W, dtype=np.float32))
    src = np.asarray(src).astype(np.int64)
    dst = np.asarray(dst).astype(np.int64)
    if n_nodes is None:
        n_nodes = x.shape[0]

    in_maps, c_list, groups, bpc, npad, qrows, perm = _preprocess(
        x, src, dst, W, n_nodes, NCORES)
    nc = _build(tuple(c_list), tuple(groups), bpc, npad, qrows)
    res = bass_utils.run_bass_kernel_spmd(
        nc, in_maps, core_ids=list(range(NCORES)), trace=trace)
    LAST_RESULTS = res
    out_perm = np.concatenate([res.results[c]["out"] for c in range(NCORES)],
                              axis=0)
    # out_perm row i corresponds to node perm[i]
    out = np.zeros((n_nodes, x.shape[1]), dtype=np.float32)
    valid = perm < n_nodes
    out[perm[valid]] = out_perm[valid]
    return out


# revision 9
# speedup vs baseline: 1.0095x; 1.0095x over previous
"""GCN layer kernel for Trainium2 (8 NeuronCores).

Strategy:
  - Nodes assigned to 8 cores x BPC blocks of 128 via balanced packing so
    each (block, src-quarter) edge segment fits C_q*128 edges (minimal pad).
  - Edge gather: x rows fetched from HBM with gpsimd.dma_gather (SWDGE),
    <=1024 indices per call (HW limit), calls spread over 4 SWDGE queues
    (parallel Q7 descriptor-gen pairs). int16 indices reach 32767 rows, so
    x is split into 4 quarters with shifted base APs.
  - Segment-sum: per 128-edge chunk, one-hot S[e, slot] built on DVE
    (is_equal vs iota, batched per block), PE matmul accumulates
    psum[feat, slot] += E_bf16^T S over the block's chunks. Pad edges get
    slot 128 (PSUM cols 128..131 ignored).
  - mean+linear+relu+residual: psum1 -> SBUF (ACT copy), f32 matmul with
    W^T, ACT relu with per-partition scale 1/deg (folds the mean), DVE
    residual add, DMA out. deg==0 nodes get a self-edge and scale 1 so
    they keep x (DGL semantics).
"""
import sys
sys.path.insert(0, "/opt/trn_rl_repo")

import numpy as np
import ml_dtypes

import concourse.bass as bass
import concourse.mybir as mybir
import concourse.tile as tile
from concourse import bacc, bass_utils

F32 = mybir.dt.float32
BF16 = mybir.dt.bfloat16
I16 = mybir.dt.int16

N_NODES = 100000
D = 128
NCORES = 8
SW = 132  # slot one-hot width (128 real slots + pad cols)
PAD_SLOT = 128
GATHER_CAP = 1024  # dma_gather crashes above 1024 indices per call

_BUILD_CACHE = {}
LAST_RESULTS = None  # for test harness introspection


def _plan_groups(bpc, gmax):
    groups = []
    left = bpc
    while left > 0:
        g = min(gmax, left)
        groups.append(g)
        left -= g
    return groups


def _build(c_list, groups, bpc, npad, qrows):
    """Build + compile the SPMD Bass program. Same program for all 8 cores."""
    key = (tuple(c_list), tuple(groups), bpc, npad, qrows)
    if key in _BUILD_CACHE:
        return _BUILD_CACHE[key]

    csum = int(sum(c_list))
    totslots = bpc * csum * 128
    npc = bpc * 128  # nodes per core

    nc = bacc.Bacc("TRN2", target_bir_lowering=False, debug=False,
                   num_devices=NCORES, num_swdge_queues=4)
    xq = nc.dram_tensor("xq", [npad, D], BF16, kind="ExternalInput")
    idxd = nc.dram_tensor("idxd", [128, totslots // 16], I16,
                          kind="ExternalInput")
    slotd = nc.dram_tensor("slotd", [128, bpc * csum], BF16,
                           kind="ExternalInput")
    invd = nc.dram_tensor("invd", [128, bpc], F32, kind="ExternalInput")
    xod = nc.dram_tensor("xod", [npc, D], F32, kind="ExternalInput")
    wtd = nc.dram_tensor("wtd", [D, D], F32, kind="ExternalInput")
    iotad = nc.dram_tensor("iotad", [128, SW], BF16, kind="ExternalInput")
    outd = nc.dram_tensor("out", [npc, D], F32, kind="ExternalOutput")

    qcall = 0  # rotates gather calls over the 4 SWDGE queues

    with tile.TileContext(nc) as tc:
        with tc.tile_pool(name="const", bufs=1) as const, \
             tc.tile_pool(name="ework", bufs=2) as ework, \
             tc.tile_pool(name="sbwork", bufs=3) as sbwork, \
             tc.tile_pool(name="psum1", bufs=2, space="PSUM") as psum1p, \
             tc.tile_pool(name="psum2", bufs=2, space="PSUM") as psum2p:

            wt_t = const.tile([128, D], F32)
            nc.sync.dma_start(out=wt_t[:], in_=wtd[:, :])
            iota_t = const.tile([128, SW], BF16)
            nc.sync.dma_start(out=iota_t[:], in_=iotad[:, :])
            inv_t = const.tile([128, bpc], F32)
            nc.sync.dma_start(out=inv_t[:], in_=invd[:, :])

            base_col = 0
            b0 = 0
            for g_i, G in enumerate(groups):
                slot_t = ework.tile([128, G * csum], BF16, tag="slot")
                nc.sync.dma_start(
                    out=slot_t[:],
                    in_=slotd[:, b0 * csum:(b0 + G) * csum])
                eb_list = []
                for q in range(4):
                    nidx = G * c_list[q] * 128
                    ncols = nidx // 16
                    idx_t = ework.tile([128, ncols], I16, tag=f"idx{q}")
                    nc.sync.dma_start(
                        out=idx_t[:],
                        in_=idxd[:, base_col:base_col + ncols])
                    ef = ework.tile([128, G * c_list[q], 128], BF16,
                                    tag=f"ef{q}")
                    c0 = 0
                    while c0 * 128 < nidx:
                        n_call = min(GATHER_CAP, nidx - c0 * 128)
                        nch = n_call // 128
                        nc.gpsimd.dma_gather(
                            out_ap=ef[:, c0:c0 + nch, :],
                            in_ap=xq[q * qrows:, :],
                            idxs_ap=idx_t[:, c0 * 8:(c0 + nch) * 8],
                            num_idxs=n_call,
                            num_idxs_reg=n_call,
                            elem_size=D,
                            queue_num=qcall % 4,
                        )
                        qcall += 1
                        c0 += nch
                    eb_list.append(ef)
                    base_col += ncols

                for bl in range(G):
                    b = b0 + bl
                    s_t = ework.tile([128, csum, SW], BF16, tag="S")
                    slot_sl = slot_t[:, bl * csum:(bl + 1) * csum]
                    nc.vector.tensor_tensor(
                        out=s_t[:, :, :],
                        in0=slot_sl.unsqueeze(2).to_broadcast(
                            [128, csum, SW]),
                        in1=iota_t[:].unsqueeze(1).to_broadcast(
                            [128, csum, SW]),
                        op=mybir.AluOpType.is_equal,
                    )
                    p1 = psum1p.tile([128, SW], F32, tag="p1")
                    cglob = 0
                    for q in range(4):
                        cq = c_list[q]
                        for k in range(cq):
                            nc.tensor.matmul(
                                out=p1[:, :],
                                lhsT=eb_list[q][:, bl * cq + k, :],
                                rhs=s_t[:, cglob, :],
                                start=(cglob == 0),
                                stop=(cglob == csum - 1),
                            )
                            cglob += 1
                    agg_t = sbwork.tile([128, 128], F32, tag="aggT")
                    nc.scalar.copy(agg_t[:], p1[:, 0:128])
                    p2 = psum2p.tile([128, 128], F32, tag="p2")
                    nc.tensor.matmul(out=p2[:, :], lhsT=agg_t[:],
                                     rhs=wt_t[:], start=True, stop=True)
                    xo_t = sbwork.tile([128, 128], F32, tag="xo")
                    nc.sync.dma_start(out=xo_t[:],
                                      in_=xod[b * 128:(b + 1) * 128, :])
                    hb_t = sbwork.tile([128, 128], F32, tag="hb")
                    nc.scalar.activation(
                        hb_t[:], p2[:, :],
                        mybir.ActivationFunctionType.Relu,
                        scale=inv_t[:, b:b + 1])
                    ob_t = sbwork.tile([128, 128], F32, tag="ob")
                    nc.vector.tensor_add(ob_t[:], hb_t[:], xo_t[:])
                    nc.sync.dma_start(out=outd[b * 128:(b + 1) * 128, :],
                                      in_=ob_t[:])
                b0 += G
    nc.compile()
    _BUILD_CACHE[key] = nc
    return nc


def _pack_blocks(qd, nblocks, cap, node_cap=128):
    """Greedy 4-D balanced packing: assign nodes to blocks so that each
    block's per-quarter edge counts stay <= cap and node count <= node_cap.
    qd: [n, 4] per-node per-quarter in-degree. Returns block id per node,
    or None if infeasible."""
    n = qd.shape[0]
    loads = np.zeros((nblocks, 4), dtype=np.int64)
    slots = np.full(nblocks, node_cap, dtype=np.int64)
    assign = np.full(n, -1, dtype=np.int64)
    order = np.argsort(-qd.sum(1), kind="stable")
    # big nodes first with exact argmin; tail nodes in bulk round-robin
    big = order[qd[order].sum(1) > 24]
    small = order[qd[order].sum(1) <= 24]
    for i in big:
        score = (loads + qd[i]).max(1)
        score[slots <= 0] = 1 << 40
        score[(loads + qd[i] > cap).any(1)] = 1 << 40
        b = int(np.argmin(score))
        if score[b] >= 1 << 40:
            return None
        assign[i] = b
        loads[b] += qd[i]
        slots[b] -= 1
    # small nodes: repeatedly place into least-loaded blocks
    for i in small:
        score = (loads + qd[i]).max(1).astype(np.float64)
        score += (node_cap - slots) * 1e-3  # prefer emptier blocks slightly
        score[slots <= 0] = 1e18
        score[(loads + qd[i] > cap).any(1)] = 1e18
        b = int(np.argmin(score))
        if score[b] >= 1e18:
            return None
        assign[i] = b
        loads[b] += qd[i]
        slots[b] -= 1
    return assign


def _preprocess(x, src, dst, W, n_nodes, ncores, gmax=8, bpc=None):
    """Host-side graph partitioning -> per-core tensors + chunk budgets."""
    D_ = x.shape[1]
    if bpc is None:
        # extra blocks beyond the minimum give the packer slack
        min_bpc = -(-n_nodes // (ncores * 128))
        bpc = min_bpc + 2 if min_bpc > 8 else min_bpc
    npc = bpc * 128
    npad = npc * ncores
    qrows = npad // 4
    nblocks = ncores * bpc

    deg = np.bincount(dst, minlength=n_nodes)
    inv = 1.0 / np.maximum(deg, 1).astype(np.float32)
    zero_deg = np.where(deg == 0)[0]
    if len(zero_deg):
        src = np.concatenate([src, zero_deg])
        dst = np.concatenate([dst, zero_deg])

    quarter = (src // qrows).astype(np.int64)

    # per-node per-quarter in-degree -> balanced packing
    qd = np.zeros((npad, 4), dtype=np.int64)
    np.add.at(qd, (dst, quarter), 1)
    assign = _pack_blocks(qd, nblocks, cap=512)
    if assign is not None:
        c_list = [4, 4, 4, 4]
    else:
        # fallback: natural-order blocks, budgets from data
        assign = np.arange(npad) // 128
        cmat = np.zeros((nblocks, 4), dtype=np.int64)
        np.add.at(cmat, (assign[dst], quarter), 1)
        c_list = [int(-(-cmat[:, q].max() // 128)) for q in range(4)]
        c_list = [max(c, 1) for c in c_list]
    csum = int(sum(c_list))
    qoff = np.concatenate([[0], np.cumsum(c_list)]).astype(int)

    # node -> (block, slot); slot = rank within block
    order_nodes = np.argsort(assign[:npad], kind="stable")
    # nodes with assign==-1 (padding, unassigned) -> fill remaining slots
    blk_of = assign[:npad].copy()
    unassigned = np.where(blk_of < 0)[0]
    if len(unassigned):
        counts = np.bincount(blk_of[blk_of >= 0], minlength=nblocks)
        free = []
        for b in range(nblocks):
            free.extend([b] * (128 - counts[b]))
        blk_of[unassigned] = np.array(free[:len(unassigned)], dtype=np.int64)
    order_nodes = np.argsort(blk_of, kind="stable")
    slot_of = np.zeros(npad, dtype=np.int64)
    counts = np.bincount(blk_of, minlength=nblocks)
    assert counts.max() <= 128, "block overflow"
    start = np.concatenate([[0], np.cumsum(counts)])
    slot_of[order_nodes] = np.arange(npad) - start[blk_of[order_nodes]]
    # perm[i] = node occupying padded position i (core-major, block, slot)
    pos_of = blk_of * 128 + slot_of
    perm = np.zeros(npad, dtype=np.int64)
    perm[pos_of] = np.arange(npad)

    groups = _plan_groups(bpc, gmax)
    call_base = np.zeros((len(groups), 4), dtype=np.int64)
    pos = 0
    for gi, G in enumerate(groups):
        for q in range(4):
            call_base[gi, q] = pos
            pos += G * c_list[q] * 128
    totslots = pos
    assert totslots == bpc * csum * 128

    g_of_block = np.zeros(bpc, dtype=np.int64)
    boff_of_block = np.zeros(bpc, dtype=np.int64)
    b = 0
    for gi, G in enumerate(groups):
        for j in range(G):
            g_of_block[b] = gi
            boff_of_block[b] = j
            b += 1

    # per-edge data
    blk_e = blk_of[dst]
    slot_e = slot_of[dst]
    order = np.lexsort((src, quarter, blk_e))
    src_s = src[order]
    q_s = quarter[order]
    blk_s = blk_e[order]
    slot_s = slot_e[order]

    seg_id = blk_s * 4 + q_s
    seg_counts = np.bincount(seg_id, minlength=nblocks * 4)
    cmat = seg_counts.reshape(nblocks, 4)
    for q in range(4):
        assert cmat[:, q].max() <= c_list[q] * 128, \
            f"quarter {q} overflow: {cmat[:, q].max()}"
    seg_start = np.concatenate([[0], np.cumsum(seg_counts)])
    rank = np.arange(len(src_s)) - seg_start[seg_id]
    core_e = blk_s // bpc
    bl_local = blk_s % bpc
    cq_e = np.array(c_list)[q_s]
    pos_e = (call_base[g_of_block[bl_local], q_s]
             + boff_of_block[bl_local] * cq_e * 128 + rank)

    idx16 = np.zeros((ncores, 16, totslots // 16), dtype=np.int16)
    idx16[core_e, pos_e % 16, pos_e // 16] = (src_s - q_s * qrows).astype(
        np.int16)
    idx_rep = np.tile(idx16, (1, 8, 1))

    slot_arr = np.full((ncores, 128, bpc * csum), PAD_SLOT, dtype=np.float32)
    chunk_in_block = qoff[q_s] + rank // 128
    slot_arr[core_e, rank % 128, bl_local * csum + chunk_in_block] = \
        slot_s.astype(np.float32)
    slot_bf = slot_arr.astype(ml_dtypes.bfloat16)

    inv_arr = np.ones((ncores, 128, bpc), dtype=np.float32)
    nodes = np.arange(n_nodes)
    inv_arr[blk_of[nodes] // bpc, slot_of[nodes], blk_of[nodes] % bpc] = inv

    xpad = np.zeros((npad, D_), dtype=np.float32)
    xpad[:n_nodes] = x
    xpad_bf = xpad.astype(ml_dtypes.bfloat16)
    # xod rows ordered by padded position: row (core, block, slot) = x[node]
    xperm = xpad[perm]

    iota = np.tile(np.arange(SW, dtype=np.float32)[None, :],
                   (128, 1)).astype(ml_dtypes.bfloat16)
    wt = np.ascontiguousarray(W.T.astype(np.float32))

    in_maps = []
    for c in range(ncores):
        in_maps.append({
            "xq": xpad_bf,
            "idxd": np.ascontiguousarray(idx_rep[c]),
            "slotd": np.ascontiguousarray(slot_bf[c]),
            "invd": np.ascontiguousarray(inv_arr[c]),
            "xod": np.ascontiguousarray(xperm[c * npc:(c + 1) * npc]),
            "wtd": wt,
            "iotad": iota,
        })
    return in_maps, c_list, groups, bpc, npad, qrows, perm


def kernel(x, src, dst, W, n_nodes=None, trace=False):
    global LAST_RESULTS
    x = np.ascontiguousarray(np.asarray(x, dtype=np.float32))
    W = np.ascontiguousarray(np.asarray(W, dtype=np.float32))
    src = np.asarray(src).astype(np.int64)
    dst = np.asarray(dst).astype(np.int64)
    if n_nodes is None:
        n_nodes = x.shape[0]

    in_maps, c_list, groups, bpc, npad, qrows, perm = _preprocess(
        x, src, dst, W, n_nodes, NCORES)
    nc = _build(tuple(c_list), tuple(groups), bpc, npad, qrows)
    res = bass_utils.run_bass_kernel_spmd(
        nc, in_maps, core_ids=list(range(NCORES)), trace=trace)
    LAST_RESULTS = res
    out_perm = np.concatenate([res.results[c]["out"] for c in range(NCORES)],
                              axis=0)
    # out_perm row i corresponds to node perm[i]
    out = np.zeros((n_nodes, x.shape[1]), dtype=np.float32)
    valid = perm < n_nodes
    out[perm[valid]] = out_perm[valid]
    return out


# revision 10
# speedup vs baseline: 1.0280x; 1.0183x over previous
"""GCN layer kernel for Trainium2 (8 NeuronCores).

Strategy:
  - Nodes assigned to 8 cores x BPC blocks of 128 via balanced packing so
    each (block, src-quarter) edge segment fits C_q*128 edges (minimal pad).
  - Edge gather: x rows fetched from HBM with gpsimd.dma_gather (SWDGE),
    <=1024 indices per call (HW limit), calls spread over 4 SWDGE queues
    (parallel Q7 descriptor-gen pairs). int16 indices reach 32767 rows, so
    x is split into 4 quarters with shifted base APs.
  - Segment-sum: per 128-edge chunk, one-hot S[e, slot] built on DVE
    (is_equal vs iota, batched per block), PE matmul accumulates
    psum[feat, slot] += E_bf16^T S over the block's chunks. Pad edges get
    slot 128 (PSUM cols 128..131 ignored).
  - mean+linear+relu+residual: psum1 -> SBUF (ACT copy), f32 matmul with
    W^T, ACT relu with per-partition scale 1/deg (folds the mean), DVE
    residual add, DMA out. deg==0 nodes get a self-edge and scale 1 so
    they keep x (DGL semantics).
"""
import sys
sys.path.insert(0, "/opt/trn_rl_repo")

import numpy as np
import ml_dtypes

import concourse.bass as bass
import concourse.mybir as mybir
import concourse.tile as tile
from concourse import bacc, bass_utils

F32 = mybir.dt.float32
BF16 = mybir.dt.bfloat16
I16 = mybir.dt.int16

N_NODES = 100000
D = 128
NCORES = 8
SW = 132  # slot one-hot width (128 real slots + pad cols)
PAD_SLOT = 128
GATHER_CAP = 1024  # dma_gather crashes above 1024 indices per call

_BUILD_CACHE = {}
LAST_RESULTS = None  # for test harness introspection


def _plan_groups(bpc, gmax):
    groups = []
    left = bpc
    while left > 0:
        g = min(gmax, left)
        groups.append(g)
        left -= g
    return groups


def _build(c_list, groups, bpc, npad, qrows):
    """Build + compile the SPMD Bass program. Same program for all 8 cores."""
    key = (tuple(c_list), tuple(groups), bpc, npad, qrows)
    if key in _BUILD_CACHE:
        return _BUILD_CACHE[key]

    csum = int(sum(c_list))
    totslots = bpc * csum * 128
    npc = bpc * 128  # nodes per core

    nc = bacc.Bacc("TRN2", target_bir_lowering=False, debug=False,
                   num_devices=NCORES, num_swdge_queues=4)
    xq = nc.dram_tensor("xq", [npad, D], BF16, kind="ExternalInput")
    idxd = nc.dram_tensor("idxd", [128, totslots // 16], I16,
                          kind="ExternalInput")
    slotd = nc.dram_tensor("slotd", [128, bpc * csum], BF16,
                           kind="ExternalInput")
    invd = nc.dram_tensor("invd", [128, bpc], F32, kind="ExternalInput")
    xod = nc.dram_tensor("xod", [npc, D], F32, kind="ExternalInput")
    wtd = nc.dram_tensor("wtd", [D, D], F32, kind="ExternalInput")
    iotad = nc.dram_tensor("iotad", [128, SW], BF16, kind="ExternalInput")
    outd = nc.dram_tensor("out", [npc, D], F32, kind="ExternalOutput")

    qcall = 0  # rotates gather calls over the 4 SWDGE queues

    with tile.TileContext(nc) as tc:
        with tc.tile_pool(name="const", bufs=1) as const, \
             tc.tile_pool(name="ework", bufs=3) as ework, \
             tc.tile_pool(name="sbwork", bufs=3) as sbwork, \
             tc.tile_pool(name="psum1", bufs=2, space="PSUM") as psum1p, \
             tc.tile_pool(name="psum2", bufs=2, space="PSUM") as psum2p:

            wt_t = const.tile([128, D], F32)
            nc.sync.dma_start(out=wt_t[:], in_=wtd[:, :])
            iota_t = const.tile([128, SW], BF16)
            nc.sync.dma_start(out=iota_t[:], in_=iotad[:, :])
            inv_t = const.tile([128, bpc], F32)
            nc.sync.dma_start(out=inv_t[:], in_=invd[:, :])

            base_col = 0
            b0 = 0
            for g_i, G in enumerate(groups):
                slot_t = ework.tile([128, G * csum], BF16, tag="slot")
                nc.sync.dma_start(
                    out=slot_t[:],
                    in_=slotd[:, b0 * csum:(b0 + G) * csum])
                eb_list = []
                for q in range(4):
                    nidx = G * c_list[q] * 128
                    ncols = nidx // 16
                    idx_t = ework.tile([128, ncols], I16, tag=f"idx{q}")
                    nc.sync.dma_start(
                        out=idx_t[:],
                        in_=idxd[:, base_col:base_col + ncols])
                    ef = ework.tile([128, G * c_list[q], 128], BF16,
                                    tag=f"ef{q}")
                    c0 = 0
                    while c0 * 128 < nidx:
                        n_call = min(GATHER_CAP, nidx - c0 * 128)
                        nch = n_call // 128
                        nc.gpsimd.dma_gather(
                            out_ap=ef[:, c0:c0 + nch, :],
                            in_ap=xq[q * qrows:, :],
                            idxs_ap=idx_t[:, c0 * 8:(c0 + nch) * 8],
                            num_idxs=n_call,
                            num_idxs_reg=n_call,
                            elem_size=D,
                            queue_num=qcall % 4,
                        )
                        qcall += 1
                        c0 += nch
                    eb_list.append(ef)
                    base_col += ncols

                for bl in range(G):
                    b = b0 + bl
                    s_t = ework.tile([128, csum, SW], BF16, tag="S")
                    slot_sl = slot_t[:, bl * csum:(bl + 1) * csum]
                    nc.vector.tensor_tensor(
                        out=s_t[:, :, :],
                        in0=slot_sl.unsqueeze(2).to_broadcast(
                            [128, csum, SW]),
                        in1=iota_t[:].unsqueeze(1).to_broadcast(
                            [128, csum, SW]),
                        op=mybir.AluOpType.is_equal,
                    )
                    p1 = psum1p.tile([128, SW], F32, tag="p1")
                    cglob = 0
                    for q in range(4):
                        cq = c_list[q]
                        for k in range(cq):
                            nc.tensor.matmul(
                                out=p1[:, :],
                                lhsT=eb_list[q][:, bl * cq + k, :],
                                rhs=s_t[:, cglob, :],
                                start=(cglob == 0),
                                stop=(cglob == csum - 1),
                            )
                            cglob += 1
                    agg_t = sbwork.tile([128, 128], F32, tag="aggT")
                    nc.scalar.copy(agg_t[:], p1[:, 0:128])
                    p2 = psum2p.tile([128, 128], F32, tag="p2")
                    nc.tensor.matmul(out=p2[:, :], lhsT=agg_t[:],
                                     rhs=wt_t[:], start=True, stop=True)
                    xo_t = sbwork.tile([128, 128], F32, tag="xo")
                    nc.sync.dma_start(out=xo_t[:],
                                      in_=xod[b * 128:(b + 1) * 128, :])
                    hb_t = sbwork.tile([128, 128], F32, tag="hb")
                    nc.scalar.activation(
                        hb_t[:], p2[:, :],
                        mybir.ActivationFunctionType.Relu,
                        scale=inv_t[:, b:b + 1])
                    ob_t = sbwork.tile([128, 128], F32, tag="ob")
                    nc.vector.tensor_add(ob_t[:], hb_t[:], xo_t[:])
                    nc.sync.dma_start(out=outd[b * 128:(b + 1) * 128, :],
                                      in_=ob_t[:])
                b0 += G
    nc.compile()
    _BUILD_CACHE[key] = nc
    return nc


def _pack_blocks(qd, nblocks, cap, node_cap=128):
    """Greedy 4-D balanced packing: assign nodes to blocks so that each
    block's per-quarter edge counts stay <= cap and node count <= node_cap.
    qd: [n, 4] per-node per-quarter in-degree. Returns block id per node,
    or None if infeasible."""
    n = qd.shape[0]
    loads = np.zeros((nblocks, 4), dtype=np.int64)
    slots = np.full(nblocks, node_cap, dtype=np.int64)
    assign = np.full(n, -1, dtype=np.int64)
    order = np.argsort(-qd.sum(1), kind="stable")
    # big nodes first with exact argmin; tail nodes in bulk round-robin
    big = order[qd[order].sum(1) > 24]
    small = order[qd[order].sum(1) <= 24]
    for i in big:
        score = (loads + qd[i]).max(1)
        score[slots <= 0] = 1 << 40
        score[(loads + qd[i] > cap).any(1)] = 1 << 40
        b = int(np.argmin(score))
        if score[b] >= 1 << 40:
            return None
        assign[i] = b
        loads[b] += qd[i]
        slots[b] -= 1
    # small nodes: repeatedly place into least-loaded blocks
    for i in small:
        score = (loads + qd[i]).max(1).astype(np.float64)
        score += (node_cap - slots) * 1e-3  # prefer emptier blocks slightly
        score[slots <= 0] = 1e18
        score[(loads + qd[i] > cap).any(1)] = 1e18
        b = int(np.argmin(score))
        if score[b] >= 1e18:
            return None
        assign[i] = b
        loads[b] += qd[i]
        slots[b] -= 1
    return assign


def _preprocess(x, src, dst, W, n_nodes, ncores, gmax=4, bpc=None):
    """Host-side graph partitioning -> per-core tensors + chunk budgets."""
    D_ = x.shape[1]
    if bpc is None:
        # extra blocks beyond the minimum give the packer slack
        min_bpc = -(-n_nodes // (ncores * 128))
        bpc = min_bpc + 2 if min_bpc > 8 else min_bpc
    npc = bpc * 128
    npad = npc * ncores
    qrows = npad // 4
    nblocks = ncores * bpc

    deg = np.bincount(dst, minlength=n_nodes)
    inv = 1.0 / np.maximum(deg, 1).astype(np.float32)
    zero_deg = np.where(deg == 0)[0]
    if len(zero_deg):
        src = np.concatenate([src, zero_deg])
        dst = np.concatenate([dst, zero_deg])

    quarter = (src // qrows).astype(np.int64)

    # per-node per-quarter in-degree -> balanced packing
    qd = np.zeros((npad, 4), dtype=np.int64)
    np.add.at(qd, (dst, quarter), 1)
    assign = _pack_blocks(qd, nblocks, cap=512)
    if assign is not None:
        c_list = [4, 4, 4, 4]
    else:
        # fallback: natural-order blocks, budgets from data
        assign = np.arange(npad) // 128
        cmat = np.zeros((nblocks, 4), dtype=np.int64)
        np.add.at(cmat, (assign[dst], quarter), 1)
        c_list = [int(-(-cmat[:, q].max() // 128)) for q in range(4)]
        c_list = [max(c, 1) for c in c_list]
    csum = int(sum(c_list))
    qoff = np.concatenate([[0], np.cumsum(c_list)]).astype(int)

    # node -> (block, slot); slot = rank within block
    order_nodes = np.argsort(assign[:npad], kind="stable")
    # nodes with assign==-1 (padding, unassigned) -> fill remaining slots
    blk_of = assign[:npad].copy()
    unassigned = np.where(blk_of < 0)[0]
    if len(unassigned):
        counts = np.bincount(blk_of[blk_of >= 0], minlength=nblocks)
        free = []
        for b in range(nblocks):
            free.extend([b] * (128 - counts[b]))
        blk_of[unassigned] = np.array(free[:len(unassigned)], dtype=np.int64)
    order_nodes = np.argsort(blk_of, kind="stable")
    slot_of = np.zeros(npad, dtype=np.int64)
    counts = np.bincount(blk_of, minlength=nblocks)
    assert counts.max() <= 128, "block overflow"
    start = np.concatenate([[0], np.cumsum(counts)])
    slot_of[order_nodes] = np.arange(npad) - start[blk_of[order_nodes]]
    # perm[i] = node occupying padded position i (core-major, block, slot)
    pos_of = blk_of * 128 + slot_of
    perm = np.zeros(npad, dtype=np.int64)
    perm[pos_of] = np.arange(npad)

    groups = _plan_groups(bpc, gmax)
    call_base = np.zeros((len(groups), 4), dtype=np.int64)
    pos = 0
    for gi, G in enumerate(groups):
        for q in range(4):
            call_base[gi, q] = pos
            pos += G * c_list[q] * 128
    totslots = pos
    assert totslots == bpc * csum * 128

    g_of_block = np.zeros(bpc, dtype=np.int64)
    boff_of_block = np.zeros(bpc, dtype=np.int64)
    b = 0
    for gi, G in enumerate(groups):
        for j in range(G):
            g_of_block[b] = gi
            boff_of_block[b] = j
            b += 1

    # per-edge data
    blk_e = blk_of[dst]
    slot_e = slot_of[dst]
    order = np.lexsort((src, quarter, blk_e))
    src_s = src[order]
    q_s = quarter[order]
    blk_s = blk_e[order]
    slot_s = slot_e[order]

    seg_id = blk_s * 4 + q_s
    seg_counts = np.bincount(seg_id, minlength=nblocks * 4)
    cmat = seg_counts.reshape(nblocks, 4)
    for q in range(4):
        assert cmat[:, q].max() <= c_list[q] * 128, \
            f"quarter {q} overflow: {cmat[:, q].max()}"
    seg_start = np.concatenate([[0], np.cumsum(seg_counts)])
    rank = np.arange(len(src_s)) - seg_start[seg_id]
    core_e = blk_s // bpc
    bl_local = blk_s % bpc
    cq_e = np.array(c_list)[q_s]
    pos_e = (call_base[g_of_block[bl_local], q_s]
             + boff_of_block[bl_local] * cq_e * 128 + rank)

    idx16 = np.zeros((ncores, 16, totslots // 16), dtype=np.int16)
    idx16[core_e, pos_e % 16, pos_e // 16] = (src_s - q_s * qrows).astype(
        np.int16)
    idx_rep = np.tile(idx16, (1, 8, 1))

    slot_arr = np.full((ncores, 128, bpc * csum), PAD_SLOT, dtype=np.float32)
    chunk_in_block = qoff[q_s] + rank // 128
    slot_arr[core_e, rank % 128, bl_local * csum + chunk_in_block] = \
        slot_s.astype(np.float32)
    slot_bf = slot_arr.astype(ml_dtypes.bfloat16)

    inv_arr = np.ones((ncores, 128, bpc), dtype=np.float32)
    nodes = np.arange(n_nodes)
    inv_arr[blk_of[nodes] // bpc, slot_of[nodes], blk_of[nodes] % bpc] = inv

    xpad = np.zeros((npad, D_), dtype=np.float32)
    xpad[:n_nodes] = x
    xpad_bf = xpad.astype(ml_dtypes.bfloat16)
    # xod rows ordered by padded position: row (core, block, slot) = x[node]
    xperm = xpad[perm]

    iota = np.tile(np.arange(SW, dtype=np.float32)[None, :],
                   (128, 1)).astype(ml_dtypes.bfloat16)
    wt = np.ascontiguousarray(W.T.astype(np.float32))

    in_maps = []
    for c in range(ncores):
        in_maps.append({
            "xq": xpad_bf,
            "idxd": np.ascontiguousarray(idx_rep[c]),
            "slotd": np.ascontiguousarray(slot_bf[c]),
            "invd": np.ascontiguousarray(inv_arr[c]),
            "xod": np.ascontiguousarray(xperm[c * npc:(c + 1) * npc]),
            "wtd": wt,
            "iotad": iota,
        })
    return in_maps, c_list, groups, bpc, npad, qrows, perm


def kernel(x, src, dst, W, n_nodes=None, trace=False):
    global LAST_RESULTS
    x = np.ascontiguousarray(np.asarray(x, dtype=np.float32))
    W = np.ascontiguousarray(np.asarray(W, dtype=np.float32))
    src = np.asarray(src).astype(np.int64)
    dst = np.asarray(dst).astype(np.int64)
    if n_nodes is None:
        n_nodes = x.shape[0]

    in_maps, c_list, groups, bpc, npad, qrows, perm = _preprocess(
        x, src, dst, W, n_nodes, NCORES)
    nc = _build(tuple(c_list), tuple(groups), bpc, npad, qrows)
    res = bass_utils.run_bass_kernel_spmd(
        nc, in_maps, core_ids=list(range(NCORES)), trace=trace)
    LAST_RESULTS = res
    out_perm = np.concatenate([res.results[c]["out"] for c in range(NCORES)],
                              axis=0)
    # out_perm row i corresponds to node perm[i]
    out = np.zeros((n_nodes, x.shape[1]), dtype=np.float32)
    valid = perm < n_nodes
    out[perm[valid]] = out_perm[valid]
    return out
